# revision 15
# baseline (speedup 1.0000x reference)
"""Trainium2 Bass kernel for nn_ModelSpo_66786741453110 (segment_reduce).

Computes, for text_vec [64,512,512] f32:
  sbj_vec[b]  = mean of text_vec[b, start_b:end_b+1, :]
  o{1,2}[b,l] = text_vec[b,l] @ W[:512] + sbj_vec[b] @ W[512:] + bias
  loss        = masked-CE(o1, obj_start) + masked-CE(o2, obj_end)   (scalar)

Sharding: pure data parallel, batch 64 -> 8 cores x 8 batches.

Per-core device program (b = local batch 0..7):
  - text cast-DMA'd (gpsimd/SWDGE f32->bf16) into natural layout [128l, 2048].
  - xbar DMA-transpose (one per b): natb [128, 2048] -> ttbk [128, 16, 128]
    where slot k = lc*4+dc holds transpose of columns [k*128,(k+1)*128).
  - side-pass matmul group (contract l, natural layout): stationary
    [128l, 108] = [span-by-batch (8) | onehot(obj_start) 50 | onehot(obj_end)
    50] -> PSUM [108, 512d]: rows 0..7 = span-sums, rows 8..107 = G with
    G[c,d] = sum_{b,l} onehot[b,l,c]*text[b,l,d], so sum T@label = <G, W>.
  - head matmuls (contract d): stationary [W_start[:512]|W_end[:512]] chunks,
    rhs = strided ttbk slots -> T^T [100c, 512l] psum per b; exp on ScalarE
    immediately (E = exp(T), fp32r).
  - tail: sbj_vec = sbj_sum/cnt -> u = sbj_vec @ W[512:] + bias (tiny f32
    matmuls) -> w = exp(u^T); per b weighted column-sum matmul S = w_sel^T @ E
    = sum_c exp(T+u); Ln with accum_out -> sum_l ln S per (b, head).
Host combines: loss = (sum ln S - <G,W> - sum cnt_label*u) / mask_sum.
"""

import os
import sys

import numpy as np

for _p in ("/opt/trn_rl_repo",):
    if _p not in sys.path and os.path.isdir(_p):
        sys.path.insert(0, _p)

import ml_dtypes  # noqa: E402
import concourse.bass as bass  # noqa: E402
import concourse.tile as tile  # noqa: E402
from concourse import bacc, mybir  # noqa: E402
from concourse.bass_utils import run_bass_kernel_spmd  # noqa: E402
from contextlib import ExitStack  # noqa: E402

B, L, D, C = 64, 512, 512, 50
NCORES = 8
BL = B // NCORES  # local batches per core = 8
NLC = L // 128  # 4 l-chunks
NDC = D // 128  # 4 d-chunks
H2 = 2 * C  # 100, both heads
NS = BL + H2  # 108 side-stationary columns
F32 = mybir.dt.float32
F32R = mybir.dt.float32r
BF16 = mybir.dt.bfloat16
BF16NP = ml_dtypes.bfloat16

_CACHE = {}


def _build_program():
    nc = bacc.Bacc(
        "TRN2",
        target_bir_lowering=False,
        debug=False,
        enable_asserts=False,
        num_devices=NCORES,
    )
    text = nc.dram_tensor("text", [BL, L, D], F32, kind="ExternalInput").ap()
    side = nc.dram_tensor("side", [128, BL * NLC * NS], BF16, kind="ExternalInput").ap()
    blobf = nc.dram_tensor("blobf", [128, 768], F32, kind="ExternalInput").ap()
    blobb = nc.dram_tensor("blobb", [128, 128 + NDC * H2], BF16, kind="ExternalInput").ap()

    g_out = nc.dram_tensor("g_out", [NS, D + 2], F32, kind="ExternalOutput").ap()

    with tile.TileContext(nc) as tc:
        with ExitStack() as octx:
            const = octx.enter_context(tc.tile_pool(name="const", bufs=1))
            ep = octx.enter_context(tc.tile_pool(name="ep", bufs=BL))
            psS = octx.enter_context(tc.tile_pool(name="psS", bufs=1, space="PSUM"))

            ps_side = psS.tile([NS, D], F32)
            e_map = {}
            NXB = BL // 2  # batches transposed via xbar DMA

            with ExitStack() as p1:
                natp = p1.enter_context(tc.tile_pool(name="nat", bufs=BL))
                ttp = p1.enter_context(tc.tile_pool(name="tt", bufs=BL))
                psH = p1.enter_context(tc.tile_pool(name="psH", bufs=2, space="PSUM"))
                psT = p1.enter_context(tc.tile_pool(name="psT", bufs=2, space="PSUM"))

                # passthrough DMAs batched before any xbar-transpose DMA:
                # mode transitions serialize the stream. b0's side chunk and
                # text land first so the side-pass starts ASAP.
                side_s = const.tile([128, BL * NLC * NS], BF16)
                nc.sync.dma_start(
                    out=side_s[:, 0 : NLC * NS], in_=side[:, 0 : NLC * NS]
                )
                nats = []
                natb = natp.tile([128, NLC * D], BF16, tag="nat")
                nc.gpsimd.dma_start(
                    out=natb.rearrange("p (lc d) -> p lc d", lc=NLC),
                    in_=text[0].rearrange("(lc p) d -> p lc d", p=128),
                )
                nats.append(natb)
                nc.sync.dma_start(
                    out=side_s[:, NLC * NS :], in_=side[:, NLC * NS :]
                )
                blobf_s = const.tile([128, 768], F32)
                nc.sync.dma_start(out=blobf_s, in_=blobf)
                blobb_s = const.tile([128, 128 + NDC * H2], BF16)
                nc.sync.dma_start(out=blobb_s, in_=blobb)
                for b in range(1, BL):
                    natb = natp.tile([128, NLC * D], BF16, tag="nat")
                    nc.gpsimd.dma_start(
                        out=natb.rearrange("p (lc d) -> p lc d", lc=NLC),
                        in_=text[b].rearrange("(lc p) d -> p lc d", p=128),
                    )
                    nats.append(natb)
                ident_b = blobb_s[:, 0:128]
                wa_s = blobb_s[:, 128 : 128 + NDC * H2]
                wb_s = blobf_s[:, 0:400]
                cntinv_s = blobf_s[0:BL, 400:401]
                ident8_s = blobf_s[0:BL, 401 : 401 + BL]
                m12pad_s = blobf_s[0:H2, 512:640]
                cntpad_s = blobf_s[0:H2, 640 : 640 + BL]
                bias2_s = blobf_s[0:1, 411 : 411 + H2]
                ones_row_s = const.tile([1, BL], F32)
                nc.vector.memset(ones_row_s, 1.0)

                tts = [None] * BL
                # side-pass + PE transposes for the non-xbar batches
                for b in range(BL):
                    natb = nats[b]
                    for lc in range(NLC):
                        t = b * NLC + lc
                        nc.tensor.matmul(
                            ps_side,
                            lhsT=side_s[:, t * NS : (t + 1) * NS],
                            rhs=natb[:, lc * D : (lc + 1) * D],
                            start=(t == 0),
                            stop=(t == BL * NLC - 1),
                        )
                    if b >= NXB:
                        ttbk = ttp.tile([128, NLC * NDC, 128], BF16, tag="tt")
                        tk4 = ttbk.rearrange("p (lc dc) l -> p lc dc l", dc=NDC)
                        for dc in range(NDC):
                            pt = psT.tile([128, L], BF16, tag="pt")
                            for lc in range(NLC):
                                nc.tensor.transpose(
                                    pt[:, lc * 128 : (lc + 1) * 128],
                                    natb.rearrange("p (lc d) -> p lc d", lc=NLC)[
                                        :, lc, dc * 128 : (dc + 1) * 128
                                    ],
                                    ident_b,
                                )
                            nc.any.tensor_copy(
                                tk4[:, :, dc, :],
                                pt.rearrange("p (lc l) -> p lc l", lc=NLC),
                            )
                        tts[b] = ttbk
                # xbar transposes, batched contiguously
                for b in range(NXB):
                    ttbk = ttp.tile([128, NLC * NDC, 128], BF16, tag="tt")
                    nc.sync.dma_start(out=ttbk, in_=nats[b], transpose=True)
                    tts[b] = ttbk
                # heads: T^T[100c, 512l] = sum_dc W''_dc.T @ textT_dc
                for b in list(range(NXB, BL)) + list(range(NXB)):
                    ph = psH.tile([H2, L], F32, tag="ph")
                    tt4 = tts[b].rearrange("p (lc dc) l -> p lc dc l", dc=NDC)
                    for dc in range(NDC):
                        nc.tensor.matmul(
                            ph,
                            lhsT=wa_s[:, dc * H2 : (dc + 1) * H2],
                            rhs=tt4[:, :, dc, :],
                            start=(dc == 0),
                            stop=(dc == NDC - 1),
                        )
                    e_b = ep.tile([H2, L], F32R, tag="E")
                    nc.scalar.activation(e_b, ph, mybir.ActivationFunctionType.Exp)
                    e_map[b] = e_b

            with ExitStack() as p2:
                psU = p2.enter_context(tc.tile_pool(name="psU", bufs=2, space="PSUM"))
                psE = p2.enter_context(tc.tile_pool(name="psE", bufs=1, space="PSUM"))

                # sbj_vec = sbj_sum / cnt  [8, 512]
                sbj = const.tile([BL, D], F32)
                nc.vector.tensor_scalar_mul(sbj, ps_side[0:BL, :], cntinv_s)
                # transpose sbj -> [128d x 4, 8]
                pstT = psU.tile([128, NDC * BL], F32, tag="u")
                for dc in range(NDC):
                    nc.tensor.transpose(
                        pstT[:, dc * BL : (dc + 1) * BL],
                        sbj[:, dc * 128 : (dc + 1) * 128],
                        ident8_s,
                    )
                sbjT_s = const.tile([128, NDC * BL], F32)
                nc.any.tensor_copy(sbjT_s, pstT)
                # uT = [W1b|W2b].T @ sbj_vec.T + bias  [100, 8]
                puT = psU.tile([H2, BL], F32, tag="u")
                for dc in range(NDC):
                    nc.tensor.matmul(
                        puT,
                        lhsT=wb_s[:, dc * H2 : (dc + 1) * H2],
                        rhs=sbjT_s[:, dc * BL : (dc + 1) * BL],
                        start=(dc == 0),
                        stop=False,
                    )
                nc.tensor.matmul(
                    puT, lhsT=bias2_s, rhs=ones_row_s, start=False, stop=True
                )
                uTS = const.tile([H2, BL], F32)
                nc.any.tensor_copy(uTS, puT)
                # w = exp(uT)  [100, 8]
                w_s = const.tile([H2, BL], F32)
                nc.scalar.activation(w_s, puT, mybir.ActivationFunctionType.Exp)

                gc = const.tile([NS, D + 2], F32)
                nc.any.tensor_copy(gc[:, 0:D], ps_side)
                # u-term: gc[c, D] = sum_b cnt_label[c,b] * uT[c,b]
                ut2 = const.tile([H2, BL], F32)
                nc.vector.tensor_mul(ut2, uTS, cntpad_s)
                nc.vector.reduce_sum(
                    gc[0:H2, D : D + 1], ut2, axis=mybir.AxisListType.X
                )

                # zero-padded per-b stationaries [100, 16]: cols 2b,2b+1 live
                wsels = const.tile([H2, BL, 2 * BL], F32R)
                m12pad4 = m12pad_s.rearrange("c (b j) -> c b j", b=BL)
                for b in range(BL):
                    nc.vector.tensor_scalar_mul(
                        wsels[:, b, :], m12pad4[:, b, :], w_s[:, b : b + 1]
                    )
                # S[2b+h, l] = sum_c w[c,h]*E_b[c,l], all 8 b accumulated;
                # late-arriving (xbar-set) E tiles go last
                ps_S = psE.tile([2 * BL, L], F32)
                e_order = list(range(NXB, BL)) + list(range(NXB))
                for i, b in enumerate(e_order):
                    nc.tensor.matmul(
                        ps_S,
                        lhsT=wsels[:, b, :],
                        rhs=e_map[b],
                        start=(i == 0),
                        stop=(i == BL - 1),
                    )
                nc.sync.dma_start(out=g_out[:, 0:D], in_=gc[:, 0:D])
                lnscr = const.tile([2 * BL, L], F32)
                nc.scalar.activation(
                    lnscr,
                    ps_S,
                    mybir.ActivationFunctionType.Ln,
                    accum_out=gc[0 : 2 * BL, D + 1 : D + 2],
                )
                nc.sync.dma_start(out=g_out[:, D:], in_=gc[:, D:])

    nc.compile()
    return nc


def _get_program():
    if "nc" not in _CACHE:
        _CACHE["nc"] = _build_program()
    return _CACHE["nc"]


def _host_prep(text_vec, sbj_bound, obj_start, obj_end, W_start, b_start, W_end, b_end):
    """Build per-core input maps."""
    text_vec = np.ascontiguousarray(np.asarray(text_vec, dtype=np.float32))
    sbj = np.asarray(sbj_bound).astype(np.int64)
    objs = np.asarray(obj_start).astype(np.int64)
    obje = np.asarray(obj_end).astype(np.int64)
    W_start = np.asarray(W_start, dtype=np.float32)
    W_end = np.asarray(W_end, dtype=np.float32)
    b_start = np.asarray(b_start, dtype=np.float32)
    b_end = np.asarray(b_end, dtype=np.float32)

    wa_cat = np.concatenate([W_start[:D], W_end[:D]], axis=1)  # [512, 100]
    wb_cat = np.concatenate([W_start[D:], W_end[D:]], axis=1)  # [512, 100]
    wa_h = np.ascontiguousarray(
        wa_cat.reshape(NDC, 128, H2).transpose(1, 0, 2).reshape(128, NDC * H2)
    ).astype(BF16NP)
    wb_h = np.ascontiguousarray(
        wb_cat.reshape(NDC, 128, H2).transpose(1, 0, 2).reshape(128, NDC * H2)
    )
    bias2 = np.concatenate([b_start, b_end]).astype(np.float32)

    blobf = np.zeros((128, 768), dtype=np.float32)
    blobf[:, 0:400] = wb_h
    blobf[0:BL, 401 : 401 + BL] = np.eye(BL, dtype=np.float32)
    blobf[:C, 409] = 1.0
    blobf[C:H2, 410] = 1.0
    blobf[0, 411 : 411 + H2] = bias2
    for b in range(BL):
        blobf[0:C, 512 + b * 16 + 2 * b] = 1.0
        blobf[C:H2, 512 + b * 16 + 2 * b + 1] = 1.0

    blobb = np.zeros((128, 128 + NDC * H2), dtype=BF16NP)
    blobb[:, 0:128] = np.eye(128, dtype=BF16NP)
    blobb[:, 128:] = wa_h

    pos = np.arange(L)
    span_all = (
        (pos[None, :] >= sbj[:, 0:1]) & (pos[None, :] <= sbj[:, 1:2])
    ).astype(np.float32)  # [B, L]
    cnt_all = span_all.sum(axis=1)  # [B]

    in_maps = []
    for c in range(NCORES):
        gb = slice(c * BL, (c + 1) * BL)
        # side stationary [t = b*4+lc][p][j]
        side_t = np.zeros((BL * NLC, 128, NS), dtype=np.float32)
        for b in range(BL):
            g = c * BL + b
            for lc in range(NLC):
                rows = slice(lc * 128, (lc + 1) * 128)
                t = b * NLC + lc
                side_t[t, :, b] = span_all[g, rows]
                ls = objs[g, rows]
                le = obje[g, rows]
                side_t[t, np.arange(128), BL + ls] = 1.0
                side_t[t, np.arange(128), BL + C + le] = 1.0
        side_h = np.ascontiguousarray(
            side_t.transpose(1, 0, 2).reshape(128, BL * NLC * NS)
        ).astype(BF16NP)
        blobf_c = blobf.copy()
        blobf_c[0:BL, 400] = (1.0 / cnt_all[gb]).astype(np.float32)
        for b in range(BL):
            g = c * BL + b
            blobf_c[0:C, 640 + b] = np.bincount(objs[g], minlength=C)
            blobf_c[C:H2, 640 + b] = np.bincount(obje[g], minlength=C)
        in_maps.append(
            {
                "text": text_vec[gb],
                "side": side_h,
                "blobf": blobf_c,
                "blobb": blobb,
            }
        )
    return in_maps


def kernel(
    text_vec,
    text_mask,
    sbj_bound,
    obj_start,
    obj_end,
    W_start,
    b_start,
    W_end,
    b_end,
):
    text_mask = np.asarray(text_mask)
    if not bool(text_mask.all()):
        # Spec guarantees all-ones mask; numpy fallback for generality.
        return _numpy_reference(
            text_vec, text_mask, sbj_bound, obj_start, obj_end,
            W_start, b_start, W_end, b_end,
        )

    nc = _get_program()
    in_maps = _host_prep(
        text_vec, sbj_bound, obj_start, obj_end, W_start, b_start, W_end, b_end
    )
    res = run_bass_kernel_spmd(nc, in_maps, core_ids=list(range(NCORES)))

    W_start = np.asarray(W_start, dtype=np.float32)
    W_end = np.asarray(W_end, dtype=np.float32)
    objs = np.asarray(obj_start).astype(np.int64)
    obje = np.asarray(obj_end).astype(np.int64)

    w1aT = W_start[:D].T.astype(np.float64)  # [50, 512]
    w2aT = W_end[:D].T.astype(np.float64)

    total = 0.0
    for c in range(NCORES):
        r = res.results[c]
        out = r["g_out"].astype(np.float64)  # [108, 514]
        g = out[BL:NS, 0:D]  # [100, 512]
        gather_t = float((g[:C] * w1aT).sum() + (g[C:] * w2aT).sum())
        u_term = float(out[0:H2, D].sum())
        ln_sum = float(out[0 : 2 * BL, D + 1].sum())
        total += ln_sum - gather_t - u_term

    value_num = float(text_mask.sum())
    return np.array(total / value_num, dtype=np.float32)


def _numpy_reference(
    text_vec, text_mask, sbj_bound, obj_start, obj_end, W_start, b_start, W_end, b_end
):
    text_vec = np.asarray(text_vec, dtype=np.float32)
    maskf = np.asarray(text_mask).astype(np.float32)
    sbj = np.asarray(sbj_bound).astype(np.int64)
    objs = np.asarray(obj_start).astype(np.int64)
    obje = np.asarray(obj_end).astype(np.int64)
    W_start = np.asarray(W_start, dtype=np.float32)
    W_end = np.asarray(W_end, dtype=np.float32)
    b_start = np.asarray(b_start, dtype=np.float32)
    b_end = np.asarray(b_end, dtype=np.float32)

    pos = np.arange(L)
    span = (
        (pos[None, :] >= sbj[:, 0:1]) & (pos[None, :] <= sbj[:, 1:2])
    ).astype(np.float32)
    count = span.sum(axis=1, keepdims=True)
    sbj_vec = np.einsum("bl,bld->bd", span, text_vec) / count

    def head(W, bv):
        return (
            np.einsum("bld,dc->blc", text_vec, W[:D]) + (sbj_vec @ W[D:])[:, None, :] + bv
        )

    def masked_ce(logits, labels, maskf, vn):
        m = logits.max(axis=-1, keepdims=True)
        logp = logits - m - np.log(np.exp(logits - m).sum(axis=-1, keepdims=True))
        nll = -np.take_along_axis(logp, labels[..., None], axis=-1)[..., 0]
        return (nll * maskf).sum() / vn

    vn = maskf.sum()
    o1 = head(W_start, b_start)
    o2 = head(W_end, b_end)
    return np.array(
        masked_ce(o1, objs, maskf, vn) + masked_ce(o2, obje, maskf, vn),
        dtype=np.float32,
    )


# revision 16
# speedup vs baseline: 1.0401x; 1.0401x over previous
"""Trainium2 Bass kernel for nn_ModelSpo_66786741453110 (segment_reduce).

Computes, for text_vec [64,512,512] f32:
  sbj_vec[b]  = mean of text_vec[b, start_b:end_b+1, :]
  o{1,2}[b,l] = text_vec[b,l] @ W[:512] + sbj_vec[b] @ W[512:] + bias
  loss        = masked-CE(o1, obj_start) + masked-CE(o2, obj_end)   (scalar)

Sharding: pure data parallel, batch 64 -> 8 cores x 8 batches.

Per-core device program (b = local batch 0..7):
  - text cast-DMA'd (gpsimd/SWDGE f32->bf16) into natural layout [128l, 2048].
  - xbar DMA-transpose (one per b): natb [128, 2048] -> ttbk [128, 16, 128]
    where slot k = lc*4+dc holds transpose of columns [k*128,(k+1)*128).
  - side-pass matmul group (contract l, natural layout): stationary
    [128l, 108] = [span-by-batch (8) | onehot(obj_start) 50 | onehot(obj_end)
    50] -> PSUM [108, 512d]: rows 0..7 = span-sums, rows 8..107 = G with
    G[c,d] = sum_{b,l} onehot[b,l,c]*text[b,l,d], so sum T@label = <G, W>.
  - head matmuls (contract d): stationary [W_start[:512]|W_end[:512]] chunks,
    rhs = strided ttbk slots -> T^T [100c, 512l] psum per b; exp on ScalarE
    immediately (E = exp(T), fp32r).
  - tail: sbj_vec = sbj_sum/cnt -> u = sbj_vec @ W[512:] + bias (tiny f32
    matmuls) -> w = exp(u^T); per b weighted column-sum matmul S = w_sel^T @ E
    = sum_c exp(T+u); Ln with accum_out -> sum_l ln S per (b, head).
Host combines: loss = (sum ln S - <G,W> - sum cnt_label*u) / mask_sum.
"""

import os
import sys

import numpy as np

for _p in ("/opt/trn_rl_repo",):
    if _p not in sys.path and os.path.isdir(_p):
        sys.path.insert(0, _p)

import ml_dtypes  # noqa: E402
import concourse.bass as bass  # noqa: E402
import concourse.tile as tile  # noqa: E402
from concourse import bacc, mybir  # noqa: E402
from concourse.bass_utils import run_bass_kernel_spmd  # noqa: E402
from contextlib import ExitStack  # noqa: E402

B, L, D, C = 64, 512, 512, 50
NCORES = 8
BL = B // NCORES  # local batches per core = 8
NLC = L // 128  # 4 l-chunks
NDC = D // 128  # 4 d-chunks
H2 = 2 * C  # 100, both heads
NS = BL + H2  # 108 side-stationary columns
F32 = mybir.dt.float32
F32R = mybir.dt.float32r
BF16 = mybir.dt.bfloat16
BF16NP = ml_dtypes.bfloat16

_CACHE = {}


def _build_program():
    nc = bacc.Bacc(
        "TRN2",
        target_bir_lowering=False,
        debug=False,
        enable_asserts=False,
        num_devices=NCORES,
    )
    text = nc.dram_tensor("text", [BL, L, D], F32, kind="ExternalInput").ap()
    side = nc.dram_tensor("side", [128, BL * NLC * NS], BF16, kind="ExternalInput").ap()
    blobf = nc.dram_tensor("blobf", [128, 768], F32, kind="ExternalInput").ap()
    blobb = nc.dram_tensor("blobb", [128, 128 + NDC * H2], BF16, kind="ExternalInput").ap()

    g_out = nc.dram_tensor("g_out", [NS, D + 1], F32, kind="ExternalOutput").ap()
    s_out = nc.dram_tensor("s_out", [2 * BL, L], F32, kind="ExternalOutput").ap()

    with tile.TileContext(nc) as tc:
        with ExitStack() as octx:
            const = octx.enter_context(tc.tile_pool(name="const", bufs=1))
            ep = octx.enter_context(tc.tile_pool(name="ep", bufs=BL))
            psS = octx.enter_context(tc.tile_pool(name="psS", bufs=1, space="PSUM"))

            ps_side = psS.tile([NS, D], F32)
            e_map = {}
            NXB = BL // 2  # batches transposed via xbar DMA

            with ExitStack() as p1:
                natp = p1.enter_context(tc.tile_pool(name="nat", bufs=BL))
                ttp = p1.enter_context(tc.tile_pool(name="tt", bufs=BL))
                psH = p1.enter_context(tc.tile_pool(name="psH", bufs=2, space="PSUM"))
                psT = p1.enter_context(tc.tile_pool(name="psT", bufs=2, space="PSUM"))
                psU = p1.enter_context(tc.tile_pool(name="psU", bufs=2, space="PSUM"))

                # passthrough DMAs batched before any xbar-transpose DMA:
                # mode transitions serialize the stream. b0's side chunk and
                # text land first so the side-pass starts ASAP.
                side_s = const.tile([128, BL * NLC * NS], BF16)
                nc.sync.dma_start(
                    out=side_s[:, 0 : NLC * NS], in_=side[:, 0 : NLC * NS]
                )
                nats = []
                natb = natp.tile([128, NLC * D], BF16, tag="nat")
                nc.gpsimd.dma_start(
                    out=natb.rearrange("p (lc d) -> p lc d", lc=NLC),
                    in_=text[0].rearrange("(lc p) d -> p lc d", p=128),
                )
                nats.append(natb)
                nc.sync.dma_start(
                    out=side_s[:, NLC * NS :], in_=side[:, NLC * NS :]
                )
                blobf_s = const.tile([128, 768], F32)
                nc.sync.dma_start(out=blobf_s, in_=blobf)
                blobb_s = const.tile([128, 128 + NDC * H2], BF16)
                nc.sync.dma_start(out=blobb_s, in_=blobb)
                for b in range(1, BL):
                    natb = natp.tile([128, NLC * D], BF16, tag="nat")
                    nc.gpsimd.dma_start(
                        out=natb.rearrange("p (lc d) -> p lc d", lc=NLC),
                        in_=text[b].rearrange("(lc p) d -> p lc d", p=128),
                    )
                    nats.append(natb)
                ident_b = blobb_s[:, 0:128]
                wa_s = blobb_s[:, 128 : 128 + NDC * H2]
                wb_s = blobf_s[:, 0:400]
                cntinv_s = blobf_s[0:BL, 400:401]
                ident8_s = blobf_s[0:BL, 401 : 401 + BL]
                m12pad_s = blobf_s[0:H2, 512:640]
                cntpad_s = blobf_s[0:H2, 640 : 640 + BL]
                bias2_s = blobf_s[0:1, 411 : 411 + H2]
                ones_row_s = const.tile([1, BL], F32)
                nc.vector.memset(ones_row_s, 1.0)

                tts = [None] * BL
                # side-pass + PE transposes for the non-xbar batches
                for b in range(BL):
                    natb = nats[b]
                    for lc in range(NLC):
                        t = b * NLC + lc
                        nc.tensor.matmul(
                            ps_side,
                            lhsT=side_s[:, t * NS : (t + 1) * NS],
                            rhs=natb[:, lc * D : (lc + 1) * D],
                            start=(t == 0),
                            stop=(t == BL * NLC - 1),
                        )
                    if b >= NXB:
                        ttbk = ttp.tile([128, NLC * NDC, 128], BF16, tag="tt")
                        tk4 = ttbk.rearrange("p (lc dc) l -> p lc dc l", dc=NDC)
                        for dc in range(NDC):
                            pt = psT.tile([128, L], BF16, tag="pt")
                            for lc in range(NLC):
                                nc.tensor.transpose(
                                    pt[:, lc * 128 : (lc + 1) * 128],
                                    natb.rearrange("p (lc d) -> p lc d", lc=NLC)[
                                        :, lc, dc * 128 : (dc + 1) * 128
                                    ],
                                    ident_b,
                                )
                            nc.any.tensor_copy(
                                tk4[:, :, dc, :],
                                pt.rearrange("p (lc l) -> p lc l", lc=NLC),
                            )
                        tts[b] = ttbk

                # ---- u-chain: ready as soon as the side-pass completes ----
                sbj = const.tile([BL, D], F32)
                nc.vector.tensor_scalar_mul(sbj, ps_side[0:BL, :], cntinv_s)
                pstT = psU.tile([128, NDC * BL], F32, tag="u")
                for dc in range(NDC):
                    nc.tensor.transpose(
                        pstT[:, dc * BL : (dc + 1) * BL],
                        sbj[:, dc * 128 : (dc + 1) * 128],
                        ident8_s,
                    )
                sbjT_s = const.tile([128, NDC * BL], F32)
                nc.any.tensor_copy(sbjT_s, pstT)
                # uT = [W1b|W2b].T @ sbj_vec.T + bias  [100, 8]
                puT = psU.tile([H2, BL], F32, tag="u")
                for dc in range(NDC):
                    nc.tensor.matmul(
                        puT,
                        lhsT=wb_s[:, dc * H2 : (dc + 1) * H2],
                        rhs=sbjT_s[:, dc * BL : (dc + 1) * BL],
                        start=(dc == 0),
                        stop=False,
                    )
                nc.tensor.matmul(
                    puT, lhsT=bias2_s, rhs=ones_row_s, start=False, stop=True
                )
                uTS = const.tile([H2, BL], F32)
                nc.any.tensor_copy(uTS, puT)
                # w = exp(uT)  [100, 8]
                w_s = const.tile([H2, BL], F32)
                nc.scalar.activation(w_s, puT, mybir.ActivationFunctionType.Exp)
                # per-b zero-padded stationaries [100, 16]: cols 2b,2b+1 live
                wsels = const.tile([H2, BL, 2 * BL], F32R)
                m12pad4 = m12pad_s.rearrange("c (b j) -> c b j", b=BL)
                for b in range(BL):
                    nc.vector.tensor_scalar_mul(
                        wsels[:, b, :], m12pad4[:, b, :], w_s[:, b : b + 1]
                    )
                # G + u-term block (shipped in the early output DMA)
                gc = const.tile([NS, D + 1], F32)
                nc.any.tensor_copy(gc[:, 0:D], ps_side)
                ut2 = const.tile([H2, BL], F32)
                nc.vector.tensor_mul(ut2, uTS, cntpad_s)
                nc.vector.reduce_sum(
                    gc[0:H2, D : D + 1], ut2, axis=mybir.AxisListType.X
                )

                # xbar transposes, batched contiguously
                for b in range(NXB):
                    ttbk = ttp.tile([128, NLC * NDC, 128], BF16, tag="tt")
                    nc.sync.dma_start(out=ttbk, in_=nats[b], transpose=True)
                    tts[b] = ttbk
                # heads: T^T[100c, 512l] = sum_dc W''_dc.T @ textT_dc
                for b in list(range(NXB, BL)) + list(range(NXB)):
                    ph = psH.tile([H2, L], F32, tag="ph")
                    tt4 = tts[b].rearrange("p (lc dc) l -> p lc dc l", dc=NDC)
                    for dc in range(NDC):
                        nc.tensor.matmul(
                            ph,
                            lhsT=wa_s[:, dc * H2 : (dc + 1) * H2],
                            rhs=tt4[:, :, dc, :],
                            start=(dc == 0),
                            stop=(dc == NDC - 1),
                        )
                    e_b = ep.tile([H2, L], F32R, tag="E")
                    nc.scalar.activation(e_b, ph, mybir.ActivationFunctionType.Exp)
                    e_map[b] = e_b

            with ExitStack() as p2:
                psE = p2.enter_context(tc.tile_pool(name="psE", bufs=1, space="PSUM"))

                nc.sync.dma_start(out=g_out, in_=gc)
                # S[2b+h, l] = sum_c w[c,h]*E_b[c,l], all 8 b accumulated;
                # late-arriving (xbar-set) E tiles go last
                ps_S = psE.tile([2 * BL, L], F32)
                e_order = list(range(NXB, BL)) + list(range(NXB))
                for i, b in enumerate(e_order):
                    nc.tensor.matmul(
                        ps_S,
                        lhsT=wsels[:, b, :],
                        rhs=e_map[b],
                        start=(i == 0),
                        stop=(i == BL - 1),
                    )
                stail = const.tile([2 * BL, L], F32)
                nc.any.tensor_copy(stail, ps_S)
                nc.sync.dma_start(out=s_out, in_=stail)

    nc.compile()
    return nc


def _get_program():
    if "nc" not in _CACHE:
        _CACHE["nc"] = _build_program()
    return _CACHE["nc"]


def _host_prep(text_vec, sbj_bound, obj_start, obj_end, W_start, b_start, W_end, b_end):
    """Build per-core input maps."""
    text_vec = np.ascontiguousarray(np.asarray(text_vec, dtype=np.float32))
    sbj = np.asarray(sbj_bound).astype(np.int64)
    objs = np.asarray(obj_start).astype(np.int64)
    obje = np.asarray(obj_end).astype(np.int64)
    W_start = np.asarray(W_start, dtype=np.float32)
    W_end = np.asarray(W_end, dtype=np.float32)
    b_start = np.asarray(b_start, dtype=np.float32)
    b_end = np.asarray(b_end, dtype=np.float32)

    wa_cat = np.concatenate([W_start[:D], W_end[:D]], axis=1)  # [512, 100]
    wb_cat = np.concatenate([W_start[D:], W_end[D:]], axis=1)  # [512, 100]
    wa_h = np.ascontiguousarray(
        wa_cat.reshape(NDC, 128, H2).transpose(1, 0, 2).reshape(128, NDC * H2)
    ).astype(BF16NP)
    wb_h = np.ascontiguousarray(
        wb_cat.reshape(NDC, 128, H2).transpose(1, 0, 2).reshape(128, NDC * H2)
    )
    bias2 = np.concatenate([b_start, b_end]).astype(np.float32)

    blobf = np.zeros((128, 768), dtype=np.float32)
    blobf[:, 0:400] = wb_h
    blobf[0:BL, 401 : 401 + BL] = np.eye(BL, dtype=np.float32)
    blobf[:C, 409] = 1.0
    blobf[C:H2, 410] = 1.0
    blobf[0, 411 : 411 + H2] = bias2
    for b in range(BL):
        blobf[0:C, 512 + b * 16 + 2 * b] = 1.0
        blobf[C:H2, 512 + b * 16 + 2 * b + 1] = 1.0

    blobb = np.zeros((128, 128 + NDC * H2), dtype=BF16NP)
    blobb[:, 0:128] = np.eye(128, dtype=BF16NP)
    blobb[:, 128:] = wa_h

    pos = np.arange(L)
    span_all = (
        (pos[None, :] >= sbj[:, 0:1]) & (pos[None, :] <= sbj[:, 1:2])
    ).astype(np.float32)  # [B, L]
    cnt_all = span_all.sum(axis=1)  # [B]

    in_maps = []
    for c in range(NCORES):
        gb = slice(c * BL, (c + 1) * BL)
        # side stationary [t = b*4+lc][p][j]
        side_t = np.zeros((BL * NLC, 128, NS), dtype=np.float32)
        for b in range(BL):
            g = c * BL + b
            for lc in range(NLC):
                rows = slice(lc * 128, (lc + 1) * 128)
                t = b * NLC + lc
                side_t[t, :, b] = span_all[g, rows]
                ls = objs[g, rows]
                le = obje[g, rows]
                side_t[t, np.arange(128), BL + ls] = 1.0
                side_t[t, np.arange(128), BL + C + le] = 1.0
        side_h = np.ascontiguousarray(
            side_t.transpose(1, 0, 2).reshape(128, BL * NLC * NS)
        ).astype(BF16NP)
        blobf_c = blobf.copy()
        blobf_c[0:BL, 400] = (1.0 / cnt_all[gb]).astype(np.float32)
        for b in range(BL):
            g = c * BL + b
            blobf_c[0:C, 640 + b] = np.bincount(objs[g], minlength=C)
            blobf_c[C:H2, 640 + b] = np.bincount(obje[g], minlength=C)
        in_maps.append(
            {
                "text": text_vec[gb],
                "side": side_h,
                "blobf": blobf_c,
                "blobb": blobb,
            }
        )
    return in_maps


def kernel(
    text_vec,
    text_mask,
    sbj_bound,
    obj_start,
    obj_end,
    W_start,
    b_start,
    W_end,
    b_end,
):
    text_mask = np.asarray(text_mask)
    if not bool(text_mask.all()):
        # Spec guarantees all-ones mask; numpy fallback for generality.
        return _numpy_reference(
            text_vec, text_mask, sbj_bound, obj_start, obj_end,
            W_start, b_start, W_end, b_end,
        )

    nc = _get_program()
    in_maps = _host_prep(
        text_vec, sbj_bound, obj_start, obj_end, W_start, b_start, W_end, b_end
    )
    res = run_bass_kernel_spmd(nc, in_maps, core_ids=list(range(NCORES)))

    W_start = np.asarray(W_start, dtype=np.float32)
    W_end = np.asarray(W_end, dtype=np.float32)
    objs = np.asarray(obj_start).astype(np.int64)
    obje = np.asarray(obj_end).astype(np.int64)

    w1aT = W_start[:D].T.astype(np.float64)  # [50, 512]
    w2aT = W_end[:D].T.astype(np.float64)

    total = 0.0
    for c in range(NCORES):
        r = res.results[c]
        out = r["g_out"].astype(np.float64)  # [108, 513]
        g = out[BL:NS, 0:D]  # [100, 512]
        gather_t = float((g[:C] * w1aT).sum() + (g[C:] * w2aT).sum())
        u_term = float(out[0:H2, D].sum())
        ln_sum = float(np.log(r["s_out"].astype(np.float64)).sum())
        total += ln_sum - gather_t - u_term

    value_num = float(text_mask.sum())
    return np.array(total / value_num, dtype=np.float32)


def _numpy_reference(
    text_vec, text_mask, sbj_bound, obj_start, obj_end, W_start, b_start, W_end, b_end
):
    text_vec = np.asarray(text_vec, dtype=np.float32)
    maskf = np.asarray(text_mask).astype(np.float32)
    sbj = np.asarray(sbj_bound).astype(np.int64)
    objs = np.asarray(obj_start).astype(np.int64)
    obje = np.asarray(obj_end).astype(np.int64)
    W_start = np.asarray(W_start, dtype=np.float32)
    W_end = np.asarray(W_end, dtype=np.float32)
    b_start = np.asarray(b_start, dtype=np.float32)
    b_end = np.asarray(b_end, dtype=np.float32)

    pos = np.arange(L)
    span = (
        (pos[None, :] >= sbj[:, 0:1]) & (pos[None, :] <= sbj[:, 1:2])
    ).astype(np.float32)
    count = span.sum(axis=1, keepdims=True)
    sbj_vec = np.einsum("bl,bld->bd", span, text_vec) / count

    def head(W, bv):
        return (
            np.einsum("bld,dc->blc", text_vec, W[:D]) + (sbj_vec @ W[D:])[:, None, :] + bv
        )

    def masked_ce(logits, labels, maskf, vn):
        m = logits.max(axis=-1, keepdims=True)
        logp = logits - m - np.log(np.exp(logits - m).sum(axis=-1, keepdims=True))
        nll = -np.take_along_axis(logp, labels[..., None], axis=-1)[..., 0]
        return (nll * maskf).sum() / vn

    vn = maskf.sum()
    o1 = head(W_start, b_start)
    o2 = head(W_end, b_end)
    return np.array(
        masked_ce(o1, objs, maskf, vn) + masked_ce(o2, obje, maskf, vn),
        dtype=np.float32,
    )
